# revision 10
# baseline (speedup 1.0000x reference)
"""DSMIL forward pass on 8 Trainium2 NeuronCores (Bass/Tile), bf16 compute.

Sharding: data-parallel over bags, each bag split across a core pair
(core 2b: instances [0:4096) of bag b, core 2b+1: [4096:8192)). ONE tiny
pair-local collective (critical-instance exchange); the final softmax
partial reduction is done on the HOST from per-core (num|den) partials.

v2 structure (vs the 154us baseline):
  - h_nat comes from XBAR DMA transposes (InstDmaTransposeAnt, one
    [128,512]->[128,4,128] instr per (chunk, d-block)) instead of 128 PE
    transposes: ~23us of PE work moves to the (idle) DMA engines.
  - class scores are computed TRANSPOSED (cls^T[2, n] = W_i^T @ h^T,
    4 512-col matmuls per chunk with a tiny W_i weight load) instead of
    128 natural-layout matmuls each paying a full 128x128 ht weight
    load (~21us -> ~7us of PE).
  - per-chunk running argmax/winner: each chunk's onehot^T row is
    XBAR-transposed to natural layout, the chunk candidate m is
    extracted with 4 matmuls riding the encoder pipeline, and a
    copy_predicated select keeps the running winner. After the last
    chunk only ~4us of work remains before the AllGather launches.
  - the q-passes (Q = q_fn(h)) are DEFERRED until after the collective
    launch so ~10us of PE work hides the AllGather latency.
  - e^T rows are computed with one 512-col matmul per chunk (q_win as a
    2-col weight load), exp'd on 2 partitions, XBAR'd to natural
    layout, and the numerator accumulates e^T @ h_nat per chunk.
  - the second collective is gone: each core ships (num|den) [C, D+1];
    the host sums the pair and divides.
"""
import numpy as np
import ml_dtypes
from contextlib import ExitStack

import concourse.bacc as bacc
import concourse.tile as tile
import concourse.mybir as mybir
import concourse.bass_isa as bass_isa

F32 = mybir.dt.float32
BF16 = mybir.dt.bfloat16
AF = mybir.ActivationFunctionType
ALU = mybir.AluOpType
bfdt = ml_dtypes.bfloat16

N_CORES = 8
B_BAGS = 4
N_FULL = 8192
N_LOC = N_FULL // 2

_cache = {}


def _build_kernel(n_cores=N_CORES, N_loc=N_LOC, I=1024, D=512, QD=128,
                  C=2, CHUNK=512, N_WARM0=32, N_WARM=16):
    NB = N_loc // 128          # n-blocks (32)
    NCH = N_loc // CHUNK       # chunks (8)
    BPC = CHUNK // 128         # n-blocks per chunk (4)
    IB = I // 128              # i-blocks (8)
    DB = D // 128              # d-blocks (4)
    assert QD == 128 and C == 2
    inv_sqrt_q = 1.0 / float(np.sqrt(QD))

    nc = bacc.Bacc("TRN2", target_bir_lowering=False, debug=False,
                   num_devices=n_cores)

    xt_d = nc.dram_tensor("xt", [NCH, 128, IB, CHUNK], BF16,
                          kind="ExternalInput")
    w_enc = nc.dram_tensor("w_enc", [128, IB, D], BF16, kind="ExternalInput")
    w_i = nc.dram_tensor("w_i", [128, DB, C], BF16, kind="ExternalInput")
    w_q1 = nc.dram_tensor("w_q1", [128, DB, QD], BF16, kind="ExternalInput")
    w_q2 = nc.dram_tensor("w_q2", [QD, QD], BF16, kind="ExternalInput")
    bias_d = nc.dram_tensor("bias", [128, DB + 2], F32, kind="ExternalInput")
    identb_d = nc.dram_tensor("identb", [128, 128], BF16,
                              kind="ExternalInput")
    identf_d = nc.dram_tensor("identf", [128, 128], F32,
                              kind="ExternalInput")
    out_d = nc.dram_tensor("out", [C, D + 1], F32, kind="ExternalOutput")

    groups = [[i, i + 1] for i in range(0, n_cores, 2)]
    PAY = 2 * DB + C           # payload cols: m^T (DB*C) | max bcast (C)

    with tile.TileContext(nc) as tc, ExitStack() as ctx:
        persist = ctx.enter_context(tc.tile_pool(name="persist", bufs=1))
        dram = ctx.enter_context(tc.tile_pool(name="dram", bufs=1,
                                              space="DRAM"))

        # ---- scratch consts on the (idle) vector queue ----
        scrap = persist.tile([128, 128], BF16)
        nc.vector.memset(scrap[:], 0.0)
        warm_in = dram.tile([1, 2], F32)
        nc.scalar.dma_start(warm_in[:], identf_d[0:1, 0:2])

        # ---- consolidated weight loads (gpsimd queue) ----
        w_enc_sb = persist.tile([128, IB, D], BF16)
        nc.gpsimd.dma_start(w_enc_sb[:, 0:IB // 2, :], w_enc[:, 0:IB // 2, :])
        nc.gpsimd.dma_start(w_enc_sb[:, IB // 2:, :], w_enc[:, IB // 2:, :])
        w_q1_sb = persist.tile([128, DB, QD], BF16)
        nc.gpsimd.dma_start(w_q1_sb[:], w_q1[:])
        w_i_sb = persist.tile([128, DB, C], BF16)
        nc.gpsimd.dma_start(w_i_sb[:], w_i[:])
        w_q2_sb = persist.tile([QD, QD], BF16)
        nc.gpsimd.dma_start(w_q2_sb[:], w_q2[:])

        # warm both collective channels (fires once weights are queued)
        warm_out = dram.tile([2, 2], F32)
        nc.gpsimd.collective_compute(
            "AllGather", ALU.bypass, replica_groups=groups,
            ins=[warm_in[:].opt()], outs=[warm_out[:].opt()])

        # ---- small consts (scalar queue) ----
        identb = persist.tile([128, 128], BF16)
        nc.scalar.dma_start(identb[:], identb_d[:])
        identf = persist.tile([128, 128], F32)
        nc.scalar.dma_start(identf[:], identf_d[:])
        bias_sb = persist.tile([128, DB + 2], F32)
        nc.scalar.dma_start(bias_sb[:], bias_d[:])

        # ---- persistent activations ----
        ht_all = persist.tile([128, NCH, DB, CHUNK], BF16)   # h^T
        h_nat = persist.tile([128, NCH, BPC, D], BF16)       # h natural
        qt = persist.tile([128, NCH, CHUNK], BF16)           # Q^T
        clsT = persist.tile([C, NCH, CHUNK], F32)            # scores^T
        ohT = persist.tile([16, NCH, CHUNK], BF16)           # onehot^T (pad)
        eT = persist.tile([16, NCH, CHUNK], BF16)            # e^T (padded)
        oh_nat = persist.tile([128, NCH, BPC, 16], BF16)
        e_nat = persist.tile([128, NCH, BPC, 16], BF16)
        m_run = persist.tile([C, D], F32)                    # running winner
        run_max = persist.tile([C, 1], F32)
        dsum = persist.tile([C, NCH], F32)                   # den partials

        # pad rows of the small-transpose sources are never read back
        # (matmuls slice [:, 0:C]), but zero them so the XBAR never moves
        # uninitialized SBUF.
        nc.vector.memset(ohT[:], 0.0)
        nc.vector.memset(eT[:], 0.0)

        # ================= phase A: encoder + clsT + running winner ====
        with (
            tc.tile_pool(name="xload", bufs=2) as xload,
            tc.tile_pool(name="wp", bufs=1, space="PSUM") as wp,
            tc.tile_pool(name="hp", bufs=2, space="PSUM") as hp,
            tc.tile_pool(name="cp", bufs=2, space="PSUM") as cp,
            tc.tile_pool(name="pmp", bufs=2, space="PSUM") as pmp,
            tc.tile_pool(name="smal", bufs=4) as smal,
        ):
            # pre-warm the PE clock gate while the first DMAs land
            pw = wp.tile([128, 128], BF16, name="pw")
            for k in range(N_WARM0):
                nc.tensor.transpose(pw[:], scrap[:], scrap[:])

            xt_tiles = []
            cmax_tiles = [None] * NCH

            def emit_m_cand(cb):
                # chunk candidate m + running-winner select; emitted one
                # chunk late so the oh XBAR latency is fully hidden
                pm = pmp.tile([C, D], F32, tag="m", name="pm")
                for nb in range(BPC):
                    nc.tensor.matmul(pm[:], oh_nat[:, cb, nb, 0:C],
                                     h_nat[:, cb, nb, :],
                                     start=(nb == 0), stop=(nb == BPC - 1))
                cmax = cmax_tiles[cb]
                if cb == 0:
                    nc.vector.tensor_copy(m_run[:], pm[:])
                    nc.vector.tensor_copy(run_max[:], cmax[:])
                else:
                    msk = smal.tile([C, 1], mybir.dt.uint8, tag="msk",
                                    name="msk")
                    nc.vector.tensor_tensor(msk[:], cmax[:], run_max[:],
                                            ALU.is_gt)
                    nc.vector.copy_predicated(
                        m_run[:], msk[:].broadcast_to([C, D]), pm[:])
                    nc.vector.copy_predicated(run_max[:], msk[:], cmax[:])

            for cb in range(NCH):
                xt_c = xload.tile([128, IB, CHUNK], BF16, tag="x", name="x")
                xt_tiles.append(xt_c)
                if cb == 0:
                    nc.sync.dma_start(xt_c[:, 0:IB // 2, :],
                                      xt_d[cb][:, 0:IB // 2, :])
                    nc.scalar.dma_start(xt_c[:, IB // 2:, :],
                                        xt_d[cb][:, IB // 2:, :])
                else:
                    nc.sync.dma_start(xt_c[:], xt_d[cb])
                # onehot XBAR of the PREVIOUS chunk (after this chunk's x
                # DMA so it never delays the x stream)
                if cb >= 1:
                    nc.sync.dma_start_transpose(oh_nat[:, cb - 1, :, :],
                                                ohT[:, cb - 1, :])

                # h^T = relu(W_enc^T @ xT) per d-block; XBAR each d-block
                # to natural layout as soon as its relu lands
                for db in range(DB):
                    ph = hp.tile([128, CHUNK], F32, tag="h", name="h")
                    for ib in range(IB):
                        nc.tensor.matmul(
                            ph[:],
                            w_enc_sb[:, ib, db * 128:(db + 1) * 128],
                            xt_c[:, ib, :],
                            start=(ib == 0), stop=(ib == IB - 1))
                    nc.scalar.activation(ht_all[:, cb, db, :], ph[:],
                                         AF.Relu,
                                         bias=bias_sb[:, db:db + 1])
                    nc.scalar.dma_start_transpose(
                        h_nat[:, cb, :, db * 128:(db + 1) * 128],
                        ht_all[:, cb, db, :])

                # cls^T: [C, 512] psum, W_i as a tiny weight load
                pc = cp.tile([C, CHUNK], F32, tag="c", name="c")
                for db in range(DB):
                    nc.tensor.matmul(pc[:], w_i_sb[:, db, :],
                                     ht_all[:, cb, db, :],
                                     start=(db == 0), stop=(db == DB - 1))
                nc.scalar.copy(clsT[:, cb, :], pc[:])
                # chunk max + onehot^T row (all on 2 partitions)
                cmax = smal.tile([C, 1], F32, tag=f"cm{cb % 2}", name="cm")
                cmax_tiles[cb] = cmax
                nc.vector.reduce_max(cmax[:], clsT[:, cb, :],
                                     axis=mybir.AxisListType.X)
                nc.vector.tensor_scalar(ohT[0:C, cb, :], clsT[:, cb, :],
                                        cmax[:], None, ALU.is_equal)
                if cb >= 1:
                    emit_m_cand(cb - 1)

            nc.sync.dma_start_transpose(oh_nat[:, NCH - 1, :, :],
                                        ohT[:, NCH - 1, :])
            emit_m_cand(NCH - 1)

        # ====== payload build + exchange; q-passes hide the AllGather ===
        with (
            tc.tile_pool(name="pt", bufs=2, space="PSUM") as pt,
            tc.tile_pool(name="px", bufs=1, space="PSUM") as px,
            tc.tile_pool(name="zp", bufs=2, space="PSUM") as zp,
            tc.tile_pool(name="qp", bufs=2, space="PSUM") as qp,
            tc.tile_pool(name="zs", bufs=2) as zs,
        ):
            # payload = [ m^T (DB x C cols) | max broadcast (C cols) ]
            pay_sb = persist.tile([128, PAY], F32)
            for db in range(DB):
                ptm = pt.tile([128, C], F32, tag="t", name="ptm")
                nc.tensor.transpose(ptm[:],
                                    m_run[:, db * 128:(db + 1) * 128],
                                    identf[0:C, 0:C])
                nc.scalar.copy(pay_sb[:, db * C:(db + 1) * C], ptm[:])
            pmx = px.tile([1, C], F32, name="pmx")
            nc.tensor.transpose(pmx[:], run_max[:], identf[0:C, 0:C])
            mx_sb = persist.tile([1, C], F32)
            nc.scalar.copy(mx_sb[:], pmx[:])
            maxb = persist.tile([128, C], F32)
            nc.gpsimd.partition_broadcast(maxb[:], mx_sb[:])
            nc.scalar.copy(pay_sb[:, DB * C:], maxb[:])

            pay1 = dram.tile([128, PAY], F32)
            nc.scalar.dma_start(pay1[:], pay_sb[:])
            gath1 = dram.tile([2 * 128, PAY], F32)
            nc.gpsimd.collective_compute(
                "AllGather", ALU.bypass, replica_groups=groups,
                ins=[pay1[:].opt()], outs=[gath1[:].opt()])

            # deferred q-passes cover the collective
            for cb in range(NCH):
                pz = zp.tile([128, CHUNK], F32, tag="z", name="z")
                for db in range(DB):
                    nc.tensor.matmul(pz[:], w_q1_sb[:, db, :],
                                     ht_all[:, cb, db, :],
                                     start=(db == 0), stop=(db == DB - 1))
                zt = zs.tile([128, CHUNK], BF16, tag="zt", name="zt")
                nc.vector.tensor_scalar(zt[:], pz[:],
                                        bias_sb[:, DB:DB + 1], 0.0,
                                        ALU.add, ALU.max)
                pq = qp.tile([128, CHUNK], F32, tag="q", name="q")
                nc.tensor.matmul(pq[:], w_q2_sb[:], zt[:], start=True,
                                 stop=True)
                nc.scalar.activation(qt[:, cb, :], pq[:], AF.Tanh,
                                     bias=bias_sb[:, DB + 1:DB + 2])

            # keep the PE clock gate warm while waiting on the collective
            pwm = px.tile([128, 128], BF16, tag="pwm", name="pwm")
            for k in range(N_WARM):
                nc.tensor.transpose(pwm[:], identb[:], identb[:])

        # ================= phase B: winner, q_fn, e, num/den ===========
        with (
            tc.tile_pool(name="pb", bufs=1, space="PSUM") as pb,
            tc.tile_pool(name="ep", bufs=2, space="PSUM") as ep,
            tc.tile_pool(name="pn", bufs=1, space="PSUM") as pn,
        ):
            g2 = persist.tile([128, 2, PAY], F32)
            nc.sync.dma_start(
                g2[:], gath1[:].rearrange("(two p) f -> p two f", p=128))

            # winner-take-all merge (identical result on both cores)
            msk2 = persist.tile([128, 1, C], mybir.dt.uint8)
            nc.vector.tensor_tensor(msk2[:, 0, :], g2[:, 0, DB * C:],
                                    g2[:, 1, DB * C:], ALU.is_ge)
            m_winT = persist.tile([128, DB, C], F32)
            nc.vector.tensor_copy(
                m_winT[:], g2[:, 1, 0:DB * C].rearrange(
                    "p (db c) -> p db c", c=C))
            nc.vector.copy_predicated(
                m_winT[:], msk2[:].broadcast_to([128, DB, C]),
                g2[:, 0, 0:DB * C].rearrange("p (db c) -> p db c", c=C))
            m_winb = persist.tile([128, DB, C], BF16)
            nc.vector.tensor_copy(m_winb[:], m_winT[:])

            # q_win = q_fn(m_win)
            pzm = pb.tile([128, C], F32, tag="pzm", name="pzm")
            for db in range(DB):
                nc.tensor.matmul(pzm[:], w_q1_sb[:, db, :],
                                 m_winb[:, db, :],
                                 start=(db == 0), stop=(db == DB - 1))
            zm = persist.tile([128, C], BF16)
            nc.scalar.activation(zm[:], pzm[:], AF.Relu,
                                 bias=bias_sb[:, DB:DB + 1])
            pqc = pb.tile([128, C], F32, tag="pqc", name="pqc")
            nc.tensor.matmul(pqc[:], w_q2_sb[:], zm[:], start=True,
                             stop=True)
            q_win = persist.tile([128, C], BF16)
            nc.scalar.activation(q_win[:], pqc[:], AF.Tanh,
                                 bias=bias_sb[:, DB + 1:DB + 2])

            # e^T rows -> exp -> XBAR to natural; numerator accumulates
            pnum = pn.tile([C, D], F32, name="pnum")

            def emit_eT(cb):
                pat = ep.tile([C, CHUNK], F32, tag="at", name="at")
                nc.tensor.matmul(pat[:], q_win[:], qt[:, cb, :],
                                 start=True, stop=True)
                nc.scalar.activation(eT[0:C, cb, :], pat[:], AF.Exp,
                                     scale=inv_sqrt_q)
                eng = nc.sync if cb % 2 == 0 else nc.scalar
                eng.dma_start_transpose(e_nat[:, cb, :, :], eT[:, cb, :])
                nc.vector.reduce_sum(dsum[:, cb:cb + 1], eT[0:C, cb, :],
                                     axis=mybir.AxisListType.X)

            emit_eT(0)
            emit_eT(1)
            for cb in range(NCH):
                if cb + 2 < NCH:
                    emit_eT(cb + 2)
                for nb in range(BPC):
                    nc.tensor.matmul(
                        pnum[:], e_nat[:, cb, nb, 0:C],
                        h_nat[:, cb, nb, :],
                        start=(cb == 0 and nb == 0),
                        stop=(cb == NCH - 1 and nb == BPC - 1))

            den = persist.tile([C, 1], F32)
            nc.vector.reduce_sum(den[:], dsum[:],
                                 axis=mybir.AxisListType.X)
            out_sb = persist.tile([C, D + 1], F32)
            nc.scalar.copy(out_sb[:, 0:D], pnum[:])
            nc.vector.tensor_copy(out_sb[:, D:D + 1], den[:])
            nc.sync.dma_start(out_d[:], out_sb[:])

    nc.compile()
    return nc


def _make_in_maps(inputs, n_cores=N_CORES, N_loc=N_LOC):
    x = np.asarray(inputs["x"], dtype=np.float32)
    B = x.shape[0]
    D = int(np.asarray(inputs["W_enc"]).shape[1])
    DB = D // 128

    def bf(a):
        return np.ascontiguousarray(np.asarray(a, np.float32).astype(bfdt))

    def blk(a, last):
        # [K, M] -> [128, K//128, M] (partition-major i-block packing)
        a = np.asarray(a, np.float32)
        return np.ascontiguousarray(
            a.reshape(-1, 128, last).transpose(1, 0, 2).astype(bfdt))

    b_enc = np.asarray(inputs["b_enc"], np.float32)
    b_q1 = np.asarray(inputs["b_q1"], np.float32)
    b_q2 = np.asarray(inputs["b_q2"], np.float32)
    bias = np.zeros((128, DB + 2), np.float32)
    bias[:, 0:DB] = b_enc.reshape(DB, 128).T
    bias[:, DB] = b_q1
    bias[:, DB + 1] = b_q2

    shared = {
        "w_enc": blk(inputs["W_enc"], D),
        "w_i": blk(inputs["W_i"], 2),
        "w_q1": blk(inputs["W_q1"], 128),
        "w_q2": bf(inputs["W_q2"]),
        "bias": bias,
        "identb": np.eye(128, dtype=np.float32).astype(bfdt),
        "identf": np.eye(128, dtype=np.float32),
    }
    xb = x.astype(bfdt)
    NCH = N_loc // 512
    in_maps = []
    for core in range(n_cores):
        bag = core // 2
        half = core % 2
        xh = xb[bag % B, half * N_loc:(half + 1) * N_loc, :]
        # chunk-major: [NCH, 128(p), IB, 512(n)] with 8KB contiguous runs
        xts = np.ascontiguousarray(
            xh.reshape(NCH, 512, -1, 128).transpose(0, 3, 2, 1))
        in_maps.append({"xt": xts, **shared})
    return in_maps


def kernel(**inputs) -> np.ndarray:
    from concourse.bass_utils import run_bass_kernel_spmd

    if "nc" not in _cache:
        _cache["nc"] = _build_kernel()
    nc = _cache["nc"]
    in_maps = _make_in_maps(inputs)
    res = run_bass_kernel_spmd(nc, in_maps, core_ids=list(range(N_CORES)))
    D = 512
    outs = []
    for b in range(B_BAGS):
        pa = res.results[2 * b]["out"].astype(np.float64)
        pb = res.results[2 * b + 1]["out"].astype(np.float64)
        num = pa[:, 0:D] + pb[:, 0:D]
        den = pa[:, D] + pb[:, D]
        outs.append(num / den[:, None])
    return np.stack(outs).astype(np.float32)
